# revision 6
# baseline (speedup 1.0000x reference)
"""KoLeo loss kernel for 8 Trainium2 NeuronCores.

Reference semantics:
    v = latents.squeeze()            # [N, D] f32, N=16384, D=64
    dp = v @ v.T ; dp[i,i] = -1      # NxN scores, diagonal excluded
    idx = argmax(dp, axis=1)         # nearest neighbor by dot product
    dist = ||v - v[idx] + 1e-6||_2
    out = mean(relu(-log(dist * N)))

Sharding: rows are block-sharded 2048/core.  Each core gets a copy of v
whose rows are ROTATED by -core*2048, so the self-match diagonal of its
local [2048, 16384] score block always lands at pair-column (row//2) --
the SPMD program is identical on all cores.

Pairwise-max trick: max(a, b) = (a + b + |a-b|) / 2.  The host ships
paired sums w = v[0::2]+v[1::2] and diffs u = v[0::2]-v[1::2]; the PE
computes dp-sums s = rows @ w.T and dp-diffs d = rows @ u.T (same FLOPs
as plain dp).  ScalarE takes |d| out of PSUM, and a fused custom
VectorE op consumes (s from PSUM, |d| from SBUF) at one output/cycle --
i.e. TWO dp elements per DVE cycle:

    pack = round_16384((s + |d|) * 8) + (pair_idx + 1);  accum = max

The fp32 magic-constant rounding makes the pack exact, so max over
packs == lexicographic (quantized pair-max, pair index) max.  The
diagonal is excluded exactly: accumulate -2^17 at the self position of
the SUM stream and -+2^17 (sign by parity) on the DIFF stream, which
turns the self-pair's max into its partner's value.

PE runs cold (1.2 GHz) on this part, so matmuls are row-packed with
tile_position: even row-tiles use array rows 0-63, odd tiles rows
64-127 (operands duplicated in SBUF partitions 64-127), two matmuls
concurrently in disjoint array quadrants.  One PSUM tile [128, 2048]
per iteration holds [A-sums | B-sums | A-diffs | B-diffs] so a single
ScalarE Abs covers both tiles' diffs.

The per-row-tile-pair tail (winner recovery, batched index gathers,
exact f32 dot tie-break with self-pair partner forcing, distance, ln,
clamp, output) is interleaved into the main loop so gathers and small
ops hide under the next pair's compute.  Host: mean of 8x2048 values.
"""

import math

import ml_dtypes
import numpy as np

N = 16384
D = 64
NCORES = 8
ROWS = N // NCORES  # 2048 rows per core
P = 128  # partitions
NT = ROWS // P  # 16 row-tiles per core
NPAIR = N // 2  # 8192 pair columns
CHUNK = 512  # pair columns per scan chunk (1 PSUM bank)
NCHUNK = NPAIR // CHUNK  # 16
BIG = 131072.0  # 2^17: diagonal suppression (exact in bf16, no f32 cancellation)

# fp32 pack constants: pack = round_16384((s+|d|)*8*16384) + pair_idx+1 (exact)
PACK_SCALE = 8.0 * 16384.0
PACK_MAGIC = 3.0 * 2.0**22 * 16384.0  # rounds to multiples of 16384
UNPACK_MAGIC = 12582912.0  # 3 * 2^22: rounds to integers

_OP_NAME = "KOLEO_PAIR_PACK_MAX"
_built = {}


def _register_pack_op():
    """Register the fused pair-max/argmax custom DVE op (idempotent)."""
    from concourse import dve_ops
    from concourse.dve_spec import (
        AluOp, C0, C1, One, Spec, Src0, Src1, Zero, lower, scan,
    )
    from concourse.dve_uop import DveOpSpec

    if _OP_NAME in dve_ops._SUB_OPCODE_FOR_NAME:
        return next(op for op in dve_ops.OPS if op.name == _OP_NAME)

    def _reference(in0, in1, s0, s1, imm2):
        p = in0.shape[0]
        s = in0.astype(np.float32).reshape(p, -1)
        a = in1.astype(np.float32).reshape(p, s.shape[1])
        z = ((s + a) * np.float32(s0) + np.float32(s1)) - np.float32(s1)
        col = (np.arange(s.shape[1], dtype=np.float32) + 1.0)[None, :]
        body = (z + col).astype(np.float32)
        acc = np.maximum(body.max(axis=-1, keepdims=True), 0.0)
        return body, acc

    body = ((Src0 + Src1) * C0 + C1 - C1) + scan(AluOp.ADD, One)
    spec = Spec(body=body, accum=AluOp.MAX, accum_init=Zero, reference=_reference)

    row = max(dve_ops._SUB_OPCODE_FOR_NAME.values()) + 1
    shas = {}
    for ver in ("v3", "v4"):
        uops = lower(spec, ver=ver)
        shas[ver] = DveOpSpec(
            name=_OP_NAME, opcode=row, uops=uops, rd1_en=True
        ).sha(ver)

    op = dve_ops.DveOp(_OP_NAME, spec, subdim=False, uops_sha=shas)
    dve_ops.OPS.append(op)
    dve_ops._SUB_OPCODE_FOR_NAME[_OP_NAME] = row
    dve_ops.CUSTOM_DVE_SPECS[_OP_NAME] = spec
    return op


def _build_nc():
    """Build + compile the per-core Bass program (same NEFF on all cores)."""
    if "nc" in _built:
        return _built["nc"]

    import concourse.bass as bass
    import concourse.mybir as mybir
    import concourse.tile as tile
    from concourse import bacc

    pack_op = _register_pack_op()

    f32 = mybir.dt.float32
    bf16 = mybir.dt.bfloat16
    i32 = mybir.dt.int32
    Alu = mybir.AluOpType
    Act = mybir.ActivationFunctionType

    nc = bacc.Bacc(None, target_bir_lowering=False)

    # w/u/vrows_t are duplicated into partitions 64-127 for row-packing
    wt_d = nc.declare_dram_parameter("wt", [P, NPAIR], bf16, isOutput=False)
    ut_d = nc.declare_dram_parameter("ut", [P, NPAIR], bf16, isOutput=False)
    vrows_t = nc.declare_dram_parameter("vrows_t", [P, ROWS], bf16, isOutput=False)
    vrows_sb = nc.declare_dram_parameter("vrows_sb", [P, NT, D], f32, isOutput=False)
    vrot = nc.declare_dram_parameter("vrot", [N, D], f32, isOutput=False)
    out_d = nc.declare_dram_parameter("out", [P, NT], f32, isOutput=True)

    neg_eye = nc.inline_tensor(
        (np.eye(P, dtype=np.float32) * -BIG).astype(ml_dtypes.bfloat16), "neg_eye"
    )
    sgn = np.where(np.arange(P) % 2 == 0, -BIG, BIG).astype(np.float32)
    alt_eye = nc.inline_tensor(
        (np.eye(P, dtype=np.float32) * sgn).astype(ml_dtypes.bfloat16), "alt_eye"
    )
    half_np = np.zeros((P, P // 2), dtype=np.float32)
    half_np[np.arange(P), np.arange(P) // 2] = 1.0
    half_eye = nc.inline_tensor(half_np.astype(ml_dtypes.bfloat16), "half_eye")
    iota_np = np.tile(np.arange(NCHUNK, dtype=np.float32), (P, NT))
    iota_c = nc.inline_tensor(iota_np, "iota_c")  # [P, NT*NCHUNK]
    rloc = (np.arange(NT)[None, :] * P + np.arange(P)[:, None]).astype(np.float32)
    selfj0_np = 2.0 * np.floor(rloc / 2.0)
    partner_np = rloc + np.where(rloc % 2 == 0, 1.0, -1.0)
    selfj0_c = nc.inline_tensor(selfj0_np.astype(np.float32), "selfj0")
    partner_c = nc.inline_tensor(partner_np.astype(np.float32), "partner")

    with tile.TileContext(nc) as tc:
        with (
            tc.tile_pool(name="consts", bufs=1) as consts,
            tc.tile_pool(name="psum", bufs=2, space="PSUM") as psum_pool,
            tc.tile_pool(name="absp", bufs=4) as absp,
            tc.tile_pool(name="junk", bufs=2) as junk_pool,
            tc.tile_pool(name="small", bufs=1) as small,
            tc.tile_pool(name="tailp", bufs=2) as tailp,
        ):
            # ---- load inputs (stationaries + first chunks first) ----
            vrt_sb = consts.tile([P, ROWS], bf16)
            nc.sync.dma_start(vrt_sb[:], vrows_t[:])
            wt_sb = consts.tile([P, NPAIR], bf16)
            ut_sb = consts.tile([P, NPAIR], bf16)
            for c in range(0, NCHUNK, 2):
                sl = slice(c * CHUNK, (c + 2) * CHUNK)
                nc.sync.dma_start(wt_sb[:, sl], wt_d[:, sl])
                nc.sync.dma_start(ut_sb[:, sl], ut_d[:, sl])
            vr_sb = consts.tile([P, NT, D], f32)
            nc.sync.dma_start(vr_sb[:], vrows_sb[:])
            negI_sb = consts.tile([P, P], bf16)
            nc.sync.dma_start(negI_sb[:], neg_eye[:])
            altI_sb = consts.tile([P, P], bf16)
            nc.sync.dma_start(altI_sb[:], alt_eye[:])
            halfI_sb = consts.tile([P, P // 2], bf16)
            nc.sync.dma_start(halfI_sb[:], half_eye[:])
            iota_sb = consts.tile([P, NT, NCHUNK], f32)
            nc.sync.dma_start(iota_sb[:], iota_c[:])
            selfj0_sb = consts.tile([P, NT], f32)
            nc.sync.dma_start(selfj0_sb[:], selfj0_c[:])
            partner_sb = consts.tile([P, NT], f32)
            nc.sync.dma_start(partner_sb[:], partner_c[:])

            bm = small.tile([P, NT, NCHUNK], f32)  # packed per-chunk maxima
            kout = small.tile([P, NT], f32)  # per-row koleo result

            def tail(s):
                """Winner recovery + distance for row-tile pair (2s, 2s+1)."""
                tA, tB = 2 * s, 2 * s + 1
                tsl = slice(tA, tB + 1)
                g2 = tailp.tile([P, 2], f32)
                nc.vector.tensor_reduce(
                    g2[:], bm[:, tsl, :], axis=mybir.AxisListType.X, op=Alu.max
                )
                eqm = tailp.tile([P, 2, NCHUNK], f32)
                for k in range(2):
                    nc.vector.tensor_scalar(
                        out=eqm[:, k, :], in0=bm[:, tA + k, :],
                        scalar1=g2[:, k : k + 1], scalar2=None, op0=Alu.is_ge,
                    )
                eqi = tailp.tile([P, 2, NCHUNK], f32)
                nc.vector.tensor_tensor(
                    out=eqi[:], in0=eqm[:], in1=iota_sb[:, tsl, :], op=Alu.mult
                )
                cstar = tailp.tile([P, 2], f32)
                nc.vector.tensor_reduce(
                    cstar[:], eqi[:], axis=mybir.AxisListType.X, op=Alu.max
                )
                # unpack local pair index (+1): idx1 = g - 16384*round(g/16384)
                u1 = tailp.tile([P, 2], f32)
                nc.scalar.activation(
                    u1[:], g2[:], Act.Copy, bias=UNPACK_MAGIC, scale=1.0 / 16384.0
                )
                u2 = tailp.tile([P, 2], f32)
                nc.scalar.activation(
                    u2[:], u1[:], Act.Copy,
                    bias=16384.0 * UNPACK_MAGIC, scale=-16384.0,
                )
                idx1 = tailp.tile([P, 2], f32)
                nc.vector.tensor_tensor(out=idx1[:], in0=g2[:], in1=u2[:], op=Alu.add)
                # j0 = 2*(cstar*CHUNK + idx1 - 1); j1 = j0 + 1
                ct = tailp.tile([P, 2], f32)
                nc.scalar.activation(ct[:], cstar[:], Act.Copy, scale=float(CHUNK))
                pairf = tailp.tile([P, 2], f32)
                nc.vector.tensor_tensor(out=pairf[:], in0=ct[:], in1=idx1[:], op=Alu.add)
                j0f = tailp.tile([P, 2], f32)
                nc.scalar.activation(j0f[:], pairf[:], Act.Copy, bias=-2.0, scale=2.0)
                j1f = tailp.tile([P, 2], f32)
                nc.scalar.activation(j1f[:], j0f[:], Act.Copy, bias=1.0)
                j0i = tailp.tile([P, 2], i32)
                nc.vector.tensor_copy(j0i[:], j0f[:])
                j1i = tailp.tile([P, 2], i32)
                nc.vector.tensor_copy(j1i[:], j1f[:])

                gat0 = tailp.tile([P, 2, D], f32)
                nc.gpsimd.indirect_dma_start(
                    out=gat0[:], out_offset=None, in_=vrot[:],
                    in_offset=bass.IndirectOffsetOnAxis(ap=j0i[:], axis=0),
                )
                gat1 = tailp.tile([P, 2, D], f32)
                nc.gpsimd.indirect_dma_start(
                    out=gat1[:], out_offset=None, in_=vrot[:],
                    in_offset=bass.IndirectOffsetOnAxis(ap=j1i[:], axis=0),
                )
                pr0 = tailp.tile([P, 2, D], f32)
                nc.vector.tensor_tensor(
                    out=pr0[:], in0=vr_sb[:, tsl, :], in1=gat0[:], op=Alu.mult
                )
                dot0 = tailp.tile([P, 2], f32)
                nc.vector.tensor_reduce(
                    dot0[:], pr0[:], axis=mybir.AxisListType.X, op=Alu.add
                )
                pr1 = tailp.tile([P, 2, D], f32)
                nc.vector.tensor_tensor(
                    out=pr1[:], in0=vr_sb[:, tsl, :], in1=gat1[:], op=Alu.mult
                )
                dot1 = tailp.tile([P, 2], f32)
                nc.vector.tensor_reduce(
                    dot1[:], pr1[:], axis=mybir.AxisListType.X, op=Alu.add
                )
                sel1 = tailp.tile([P, 2], f32)  # 1.0 if dot1 > dot0
                nc.vector.tensor_tensor(
                    out=sel1[:], in0=dot1[:], in1=dot0[:], op=Alu.is_gt
                )
                jsel = tailp.tile([P, 2], f32)
                nc.vector.tensor_tensor(out=jsel[:], in0=j0f[:], in1=sel1[:], op=Alu.add)
                # if the winning pair is the self-pair, force the partner
                meq = tailp.tile([P, 2], f32)
                nc.vector.tensor_tensor(
                    out=meq[:], in0=j0f[:], in1=selfj0_sb[:, tsl], op=Alu.is_equal
                )
                dpar = tailp.tile([P, 2], f32)
                nc.vector.tensor_tensor(
                    out=dpar[:], in0=partner_sb[:, tsl], in1=jsel[:], op=Alu.subtract
                )
                dfix = tailp.tile([P, 2], f32)
                nc.vector.tensor_tensor(
                    out=dfix[:], in0=dpar[:], in1=meq[:], op=Alu.mult
                )
                jff = tailp.tile([P, 2], f32)
                nc.vector.tensor_tensor(out=jff[:], in0=jsel[:], in1=dfix[:], op=Alu.add)
                jfi = tailp.tile([P, 2], i32)
                nc.vector.tensor_copy(jfi[:], jff[:])
                gatf = tailp.tile([P, 2, D], f32)
                nc.gpsimd.indirect_dma_start(
                    out=gatf[:], out_offset=None, in_=vrot[:],
                    in_offset=bass.IndirectOffsetOnAxis(ap=jfi[:], axis=0),
                )
                diff = tailp.tile([P, 2, D], f32)
                nc.vector.tensor_tensor(
                    out=diff[:], in0=vr_sb[:, tsl, :], in1=gatf[:], op=Alu.subtract
                )
                diff2 = tailp.tile([P, 2, D], f32)
                nc.scalar.activation(diff2[:], diff[:], Act.Copy, bias=1.0e-6)
                dsq = tailp.tile([P, 2, D], f32)
                nc.vector.tensor_tensor(
                    out=dsq[:], in0=diff2[:], in1=diff2[:], op=Alu.mult
                )
                s2 = tailp.tile([P, 2], f32)
                nc.vector.tensor_reduce(
                    s2[:], dsq[:], axis=mybir.AxisListType.X, op=Alu.add
                )
                lns = tailp.tile([P, 2], f32)
                nc.scalar.activation(lns[:], s2[:], Act.Ln)
                # koleo = -0.5*ln(s2) - ln(N); clamp at 0
                kol = tailp.tile([P, 2], f32)
                nc.scalar.activation(
                    kol[:], lns[:], Act.Copy, bias=-math.log(float(N)), scale=-0.5
                )
                if DEBUG_NO_CLAMP:
                    nc.vector.tensor_copy(kout[:, tsl], kol[:])
                else:
                    nc.vector.tensor_scalar(
                        out=kout[:, tsl], in0=kol[:], scalar1=0.0, scalar2=None,
                        op0=Alu.max,
                    )

            # ---- main loop: row-tile pairs (tA rows 0-63, tB rows 64-127) ----
            for s in range(NT // 2):
                tA, tB = 2 * s, 2 * s + 1
                lhsA = vrt_sb[0:64, tA * P : (tA + 1) * P]
                lhsB = vrt_sb[64:128, tB * P : (tB + 1) * P]
                for c in range(NCHUNK):
                    sl = slice(c * CHUNK, (c + 1) * CHUNK)
                    big = psum_pool.tile([P, 4 * CHUNK], f32)
                    dA = c == tA // 8
                    dB = c == tB // 8
                    offA = (tA % 8) * 64
                    offB = (tB % 8) * 64
                    nc.tensor.matmul(
                        big[:, 0:512], lhsA, wt_sb[0:64, sl], start=True, stop=not dA
                    )
                    nc.tensor.matmul(
                        big[:, 512:1024], lhsB, wt_sb[64:128, sl],
                        start=True, stop=not dB,
                    )
                    nc.tensor.matmul(
                        big[:, 1024:1536], lhsA, ut_sb[0:64, sl],
                        start=True, stop=not dA,
                    )
                    nc.tensor.matmul(
                        big[:, 1536:2048], lhsB, ut_sb[64:128, sl],
                        start=True, stop=not dB,
                    )
                    if dA:
                        nc.tensor.matmul(
                            big[:, offA : offA + 64], negI_sb[:], halfI_sb[:],
                            start=False, stop=True,
                        )
                        nc.tensor.matmul(
                            big[:, 1024 + offA : 1024 + offA + 64],
                            altI_sb[:], halfI_sb[:], start=False, stop=True,
                        )
                    if dB:
                        nc.tensor.matmul(
                            big[:, 512 + offB : 512 + offB + 64],
                            negI_sb[:], halfI_sb[:], start=False, stop=True,
                        )
                        nc.tensor.matmul(
                            big[:, 1536 + offB : 1536 + offB + 64],
                            altI_sb[:], halfI_sb[:], start=False, stop=True,
                        )
                    ad = absp.tile([P, 2 * CHUNK], f32)
                    nc.scalar.activation(ad[:], big[:, 1024:2048], Act.Abs)
                    junkA = junk_pool.tile([P, CHUNK], f32)
                    nc.vector._custom_dve(
                        pack_op, out=junkA[:], in0=big[:, 0:512], in1=ad[:, 0:512],
                        s0=PACK_SCALE, s1=PACK_MAGIC,
                        accum_out=bm[:, tA, c : c + 1],
                    )
                    junkB = junk_pool.tile([P, CHUNK], f32)
                    nc.vector._custom_dve(
                        pack_op, out=junkB[:], in0=big[:, 512:1024],
                        in1=ad[:, 512:1024],
                        s0=PACK_SCALE, s1=PACK_MAGIC,
                        accum_out=bm[:, tB, c : c + 1],
                    )
                if s > 0:
                    tail(s - 1)
            tail(NT // 2 - 1)

            nc.sync.dma_start(out_d[:], kout[:])

    nc.compile()
    _built["nc"] = nc
    return nc


def _prep_in_maps(v: np.ndarray) -> list[dict]:
    bf = ml_dtypes.bfloat16
    in_maps = []
    for c in range(NCORES):
        vr = np.roll(v, -c * ROWS, axis=0)
        w = vr[0::2] + vr[1::2]  # [NPAIR, D] f32
        u = vr[0::2] - vr[1::2]
        rows = v[c * ROWS : (c + 1) * ROWS]
        wt = np.ascontiguousarray(w.T).astype(bf)
        ut = np.ascontiguousarray(u.T).astype(bf)
        rt = np.ascontiguousarray(rows.T).astype(bf)
        in_maps.append(
            {
                "wt": np.concatenate([wt, wt], axis=0),
                "ut": np.concatenate([ut, ut], axis=0),
                "vrows_t": np.concatenate([rt, rt], axis=0),
                "vrows_sb": np.ascontiguousarray(
                    rows.reshape(NT, P, D).transpose(1, 0, 2)
                ),
                "vrot": np.ascontiguousarray(vr),
            }
        )
    return in_maps


# test.py can flip these to profile the run
TRACE = False
DEBUG_NO_CLAMP = False
LAST_RESULT = {}


def kernel(latents: np.ndarray) -> np.ndarray:
    from concourse.bass_utils import run_bass_kernel_spmd

    v = np.asarray(latents, dtype=np.float32).reshape(N, D)
    nc = _build_nc()
    in_maps = _prep_in_maps(v)

    kwargs = {}
    if TRACE:
        kwargs = dict(trace=True, stitch_traces=False)
    res = run_bass_kernel_spmd(nc, in_maps, core_ids=list(range(NCORES)), **kwargs)
    LAST_RESULT["res"] = res

    vals = np.concatenate([r["out"].reshape(-1) for r in res.results])
    return np.array(np.mean(vals), dtype=np.float32)


# revision 7
# speedup vs baseline: 1.3661x; 1.3661x over previous
"""KoLeo loss kernel for 8 Trainium2 NeuronCores.

Reference semantics:
    v = latents.squeeze()            # [N, D] f32, N=16384, D=64
    dp = v @ v.T ; dp[i,i] = -1      # NxN scores, diagonal excluded
    idx = argmax(dp, axis=1)         # nearest neighbor by dot product
    dist = ||v - v[idx] + 1e-6||_2
    out = mean(relu(-log(dist * N)))

Sharding: rows are block-sharded 2048/core.  Each core gets a copy of v
whose rows are ROTATED by -core*2048, so the self-match diagonal of its
local [2048, 16384] score block always lands at pair-column (row//2) --
the SPMD program is identical on all cores.

Pairwise-max trick: max(a, b) = (a + b + |a-b|) / 2.  The host ships
paired sums w = v[0::2]+v[1::2] and diffs u = v[0::2]-v[1::2]; the PE
computes dp-sums s = rows @ w.T and dp-diffs d = rows @ u.T (same FLOPs
as plain dp).  ScalarE takes |d| out of PSUM (its only job), and a
fused custom VectorE op consumes (s from PSUM, |d| from SBUF) at one
output/cycle -- i.e. TWO dp elements per DVE cycle:

    pack = round_16384((s + |d|) * 8) + (pair_idx + 1);  accum = max

The fp32 magic-constant rounding makes the pack exact, so max over
packs == lexicographic (quantized pair-max, pair index) max.  The
diagonal is excluded exactly: accumulate -2^17 at the self position of
the SUM stream and -+2^17 (sign by parity) on the DIFF stream, which
turns the self-pair's max into its partner's value.

PE runs cold (1.2 GHz) on this part, so matmuls are row-packed with
tile_position: even row-tiles use array rows 0-63, odd tiles rows
64-127 (operands duplicated in SBUF partitions 64-127), two matmuls
concurrently in disjoint array quadrants.

Tail: recover the winning pair per row, gather both pair members with
batched indirect DMAs, pick the larger exact f32 dot (partner forced
when the winning pair is the self-pair), exact f32 distance, ln, clamp,
DMA out.  Host: mean of the 8x2048 per-row values.
"""

import math

import ml_dtypes
import numpy as np

N = 16384
D = 64
NCORES = 8
ROWS = N // NCORES  # 2048 rows per core
P = 128  # partitions
NT = ROWS // P  # 16 row-tiles per core
NPAIR = N // 2  # 8192 pair columns
CHUNK = 512  # pair columns per scan chunk (1 PSUM bank)
NCHUNK = NPAIR // CHUNK  # 16
BIG = 131072.0  # 2^17: diagonal suppression (exact in bf16, no f32 cancellation)

# fp32 pack constants: pack = round_16384((s+|d|)*8*16384) + pair_idx+1 (exact)
PACK_SCALE = 8.0 * 16384.0
PACK_MAGIC = 3.0 * 2.0**22 * 16384.0  # rounds to multiples of 16384
UNPACK_MAGIC = 12582912.0  # 3 * 2^22: rounds to integers

_OP_NAME = "KOLEO_PAIR_PACK_MAX"
_built = {}


def _register_pack_op():
    """Register the fused pair-max/argmax custom DVE op (idempotent)."""
    from concourse import dve_ops
    from concourse.dve_spec import (
        AluOp, C0, C1, One, Spec, Src0, Src1, Zero, lower, scan,
    )
    from concourse.dve_uop import DveOpSpec

    if _OP_NAME in dve_ops._SUB_OPCODE_FOR_NAME:
        return next(op for op in dve_ops.OPS if op.name == _OP_NAME)

    def _reference(in0, in1, s0, s1, imm2):
        p = in0.shape[0]
        s = in0.astype(np.float32).reshape(p, -1)
        a = in1.astype(np.float32).reshape(p, s.shape[1])
        z = ((s + a) * np.float32(s0) + np.float32(s1)) - np.float32(s1)
        col = (np.arange(s.shape[1], dtype=np.float32) + 1.0)[None, :]
        body = (z + col).astype(np.float32)
        acc = np.maximum(body.max(axis=-1, keepdims=True), 0.0)
        return body, acc

    body = ((Src0 + Src1) * C0 + C1 - C1) + scan(AluOp.ADD, One)
    spec = Spec(body=body, accum=AluOp.MAX, accum_init=Zero, reference=_reference)

    row = max(dve_ops._SUB_OPCODE_FOR_NAME.values()) + 1
    shas = {}
    for ver in ("v3", "v4"):
        uops = lower(spec, ver=ver)
        shas[ver] = DveOpSpec(
            name=_OP_NAME, opcode=row, uops=uops, rd1_en=True
        ).sha(ver)

    op = dve_ops.DveOp(_OP_NAME, spec, subdim=False, uops_sha=shas)
    dve_ops.OPS.append(op)
    dve_ops._SUB_OPCODE_FOR_NAME[_OP_NAME] = row
    dve_ops.CUSTOM_DVE_SPECS[_OP_NAME] = spec
    return op


def _build_nc():
    """Build + compile the per-core Bass program (same NEFF on all cores)."""
    if "nc" in _built:
        return _built["nc"]

    import concourse.bass as bass
    import concourse.mybir as mybir
    import concourse.tile as tile
    from concourse import bacc

    pack_op = _register_pack_op()

    f32 = mybir.dt.float32
    bf16 = mybir.dt.bfloat16
    i32 = mybir.dt.int32
    Alu = mybir.AluOpType
    Act = mybir.ActivationFunctionType

    nc = bacc.Bacc(None, target_bir_lowering=False)

    # w/u/vrows_t are duplicated into partitions 64-127 for row-packing
    wt_d = nc.declare_dram_parameter("wt", [P, NPAIR], bf16, isOutput=False)
    ut_d = nc.declare_dram_parameter("ut", [P, NPAIR], bf16, isOutput=False)
    vrows_t = nc.declare_dram_parameter("vrows_t", [P, ROWS], bf16, isOutput=False)
    vrows_sb = nc.declare_dram_parameter("vrows_sb", [P, NT, D], f32, isOutput=False)
    vrot = nc.declare_dram_parameter("vrot", [N, D], f32, isOutput=False)
    out_d = nc.declare_dram_parameter("out", [P, NT], f32, isOutput=True)

    neg_eye = nc.inline_tensor(
        (np.eye(P, dtype=np.float32) * -BIG).astype(ml_dtypes.bfloat16), "neg_eye"
    )
    sgn = np.where(np.arange(P) % 2 == 0, -BIG, BIG).astype(np.float32)
    alt_eye = nc.inline_tensor(
        (np.eye(P, dtype=np.float32) * sgn).astype(ml_dtypes.bfloat16), "alt_eye"
    )
    half_np = np.zeros((P, P // 2), dtype=np.float32)
    half_np[np.arange(P), np.arange(P) // 2] = 1.0
    half_eye = nc.inline_tensor(half_np.astype(ml_dtypes.bfloat16), "half_eye")
    iota_np = np.tile(np.arange(NCHUNK, dtype=np.float32), (P, NT))
    iota_c = nc.inline_tensor(iota_np, "iota_c")  # [P, NT*NCHUNK]
    rloc = (np.arange(NT)[None, :] * P + np.arange(P)[:, None]).astype(np.float32)
    selfj0_np = 2.0 * np.floor(rloc / 2.0)
    partner_np = rloc + np.where(rloc % 2 == 0, 1.0, -1.0)
    selfj0_c = nc.inline_tensor(selfj0_np.astype(np.float32), "selfj0")
    partner_c = nc.inline_tensor(partner_np.astype(np.float32), "partner")

    with tile.TileContext(nc) as tc:
        with (
            tc.tile_pool(name="consts", bufs=1) as consts,
            tc.tile_pool(name="psum", bufs=2, space="PSUM") as psum_pool,
            tc.tile_pool(name="absp", bufs=4) as absp,
            tc.tile_pool(name="junk", bufs=2) as junk_pool,
            tc.tile_pool(name="small", bufs=1) as small,
        ):
            # ---- load inputs: small consts + stationaries first so chunk 0
            # (which carries the diagonal-mask matmuls) can start immediately
            negI_sb = consts.tile([P, P], bf16)
            nc.sync.dma_start(negI_sb[:], neg_eye[:])
            altI_sb = consts.tile([P, P], bf16)
            nc.sync.dma_start(altI_sb[:], alt_eye[:])
            halfI_sb = consts.tile([P, P // 2], bf16)
            nc.sync.dma_start(halfI_sb[:], half_eye[:])
            vrt_sb = consts.tile([P, ROWS], bf16)
            nc.sync.dma_start(vrt_sb[:], vrows_t[:])
            wt_sb = consts.tile([P, NPAIR], bf16)
            ut_sb = consts.tile([P, NPAIR], bf16)
            for c in range(0, NCHUNK, 4):
                sl = slice(c * CHUNK, (c + 4) * CHUNK)
                nc.sync.dma_start(wt_sb[:, sl], wt_d[:, sl])
                nc.sync.dma_start(ut_sb[:, sl], ut_d[:, sl])
            vr_sb = consts.tile([P, NT, D], f32)
            nc.sync.dma_start(vr_sb[:], vrows_sb[:])
            iota_sb = consts.tile([P, NT, NCHUNK], f32)
            nc.sync.dma_start(iota_sb[:], iota_c[:])
            selfj0_sb = consts.tile([P, NT], f32)
            nc.sync.dma_start(selfj0_sb[:], selfj0_c[:])
            partner_sb = consts.tile([P, NT], f32)
            nc.sync.dma_start(partner_sb[:], partner_c[:])

            bm = small.tile([P, NT, NCHUNK], f32)  # packed per-chunk maxima

            # ---- main loop: row-tile pairs (tA rows 0-63, tB rows 64-127) ----
            for s in range(NT // 2):
                tA, tB = 2 * s, 2 * s + 1
                lhsA = vrt_sb[0:64, tA * P : (tA + 1) * P]
                lhsB = vrt_sb[64:128, tB * P : (tB + 1) * P]
                for c in range(NCHUNK):
                    sl = slice(c * CHUNK, (c + 1) * CHUNK)
                    psA = psum_pool.tile([P, CHUNK], f32)
                    pdA = psum_pool.tile([P, CHUNK], f32)
                    psB = psum_pool.tile([P, CHUNK], f32)
                    pdB = psum_pool.tile([P, CHUNK], f32)
                    dA = c == tA // 8
                    dB = c == tB // 8
                    offA = (tA % 8) * 64
                    offB = (tB % 8) * 64
                    nc.tensor.matmul(
                        psA[:], lhsA, wt_sb[0:64, sl], start=True, stop=not dA
                    )
                    nc.tensor.matmul(
                        psB[:], lhsB, wt_sb[64:128, sl], start=True, stop=not dB
                    )
                    nc.tensor.matmul(
                        pdA[:], lhsA, ut_sb[0:64, sl], start=True, stop=not dA
                    )
                    nc.tensor.matmul(
                        pdB[:], lhsB, ut_sb[64:128, sl], start=True, stop=not dB
                    )
                    if dA:
                        nc.tensor.matmul(
                            psA[:, offA : offA + 64], negI_sb[:], halfI_sb[:],
                            start=False, stop=True,
                        )
                        nc.tensor.matmul(
                            pdA[:, offA : offA + 64], altI_sb[:], halfI_sb[:],
                            start=False, stop=True,
                        )
                    if dB:
                        nc.tensor.matmul(
                            psB[:, offB : offB + 64], negI_sb[:], halfI_sb[:],
                            start=False, stop=True,
                        )
                        nc.tensor.matmul(
                            pdB[:, offB : offB + 64], altI_sb[:], halfI_sb[:],
                            start=False, stop=True,
                        )
                    adA = absp.tile([P, CHUNK], f32)
                    nc.scalar.activation(adA[:], pdA[:], Act.Abs)
                    junkA = junk_pool.tile([P, CHUNK], f32)
                    nc.vector._custom_dve(
                        pack_op, out=junkA[:], in0=psA[:], in1=adA[:],
                        s0=PACK_SCALE, s1=PACK_MAGIC,
                        accum_out=bm[:, tA, c : c + 1],
                    )
                    adB = absp.tile([P, CHUNK], f32)
                    nc.scalar.activation(adB[:], pdB[:], Act.Abs)
                    junkB = junk_pool.tile([P, CHUNK], f32)
                    nc.vector._custom_dve(
                        pack_op, out=junkB[:], in0=psB[:], in1=adB[:],
                        s0=PACK_SCALE, s1=PACK_MAGIC,
                        accum_out=bm[:, tB, c : c + 1],
                    )

            # ---- winner per row: chunk + local pair index ----
            g = small.tile([P, NT], f32)
            nc.vector.tensor_reduce(g[:], bm[:], axis=mybir.AxisListType.X, op=Alu.max)

            eqm = small.tile([P, NT, NCHUNK], f32)
            for t in range(NT):
                nc.vector.tensor_scalar(
                    out=eqm[:, t, :], in0=bm[:, t, :],
                    scalar1=g[:, t : t + 1], scalar2=None, op0=Alu.is_ge,
                )
            eqi = small.tile([P, NT, NCHUNK], f32)
            nc.vector.tensor_tensor(out=eqi[:], in0=eqm[:], in1=iota_sb[:], op=Alu.mult)
            cstar = small.tile([P, NT], f32)
            nc.vector.tensor_reduce(
                cstar[:], eqi[:], axis=mybir.AxisListType.X, op=Alu.max
            )

            # unpack local pair index (+1) from g (idx <= 512 so no round-up)
            u1 = small.tile([P, NT], f32)
            nc.scalar.activation(
                u1[:], g[:], Act.Copy, bias=UNPACK_MAGIC, scale=1.0 / 16384.0
            )
            u2 = small.tile([P, NT], f32)
            nc.scalar.activation(
                u2[:], u1[:], Act.Copy, bias=16384.0 * UNPACK_MAGIC, scale=-16384.0
            )
            idx1 = small.tile([P, NT], f32)
            nc.vector.tensor_tensor(out=idx1[:], in0=g[:], in1=u2[:], op=Alu.add)

            # j0 = 2*(cstar*CHUNK + idx1 - 1); j1 = j0 + 1
            ct = small.tile([P, NT], f32)
            nc.scalar.activation(ct[:], cstar[:], Act.Copy, scale=float(CHUNK))
            pairf = small.tile([P, NT], f32)
            nc.vector.tensor_tensor(out=pairf[:], in0=ct[:], in1=idx1[:], op=Alu.add)
            j0f = small.tile([P, NT], f32)
            nc.scalar.activation(j0f[:], pairf[:], Act.Copy, bias=-2.0, scale=2.0)
            j1f = small.tile([P, NT], f32)
            nc.scalar.activation(j1f[:], j0f[:], Act.Copy, bias=1.0)
            j0i = small.tile([P, NT], i32)
            nc.vector.tensor_copy(j0i[:], j0f[:])
            j1i = small.tile([P, NT], i32)
            nc.vector.tensor_copy(j1i[:], j1f[:])

            # ---- batched gathers of both pair members, larger exact dot ----
            gat0 = small.tile([P, NT, D], f32)
            nc.gpsimd.indirect_dma_start(
                out=gat0[:], out_offset=None, in_=vrot[:],
                in_offset=bass.IndirectOffsetOnAxis(ap=j0i[:], axis=0),
            )
            gat1 = small.tile([P, NT, D], f32)
            nc.gpsimd.indirect_dma_start(
                out=gat1[:], out_offset=None, in_=vrot[:],
                in_offset=bass.IndirectOffsetOnAxis(ap=j1i[:], axis=0),
            )

            pr0 = small.tile([P, NT, D], f32)
            nc.vector.tensor_tensor(out=pr0[:], in0=vr_sb[:], in1=gat0[:], op=Alu.mult)
            dot0 = small.tile([P, NT], f32)
            nc.vector.tensor_reduce(
                dot0[:], pr0[:], axis=mybir.AxisListType.X, op=Alu.add
            )
            pr1 = small.tile([P, NT, D], f32)
            nc.vector.tensor_tensor(out=pr1[:], in0=vr_sb[:], in1=gat1[:], op=Alu.mult)
            dot1 = small.tile([P, NT], f32)
            nc.vector.tensor_reduce(
                dot1[:], pr1[:], axis=mybir.AxisListType.X, op=Alu.add
            )
            sel1 = small.tile([P, NT], f32)  # 1.0 if dot1 > dot0
            nc.vector.tensor_tensor(out=sel1[:], in0=dot1[:], in1=dot0[:], op=Alu.is_gt)
            jsel = small.tile([P, NT], f32)
            nc.vector.tensor_tensor(out=jsel[:], in0=j0f[:], in1=sel1[:], op=Alu.add)

            # if the winning pair is the self-pair, force the partner
            meq = small.tile([P, NT], f32)
            nc.vector.tensor_tensor(
                out=meq[:], in0=j0f[:], in1=selfj0_sb[:], op=Alu.is_equal
            )
            dpar = small.tile([P, NT], f32)
            nc.vector.tensor_tensor(
                out=dpar[:], in0=partner_sb[:], in1=jsel[:], op=Alu.subtract
            )
            dfix = small.tile([P, NT], f32)
            nc.vector.tensor_tensor(out=dfix[:], in0=dpar[:], in1=meq[:], op=Alu.mult)
            jff = small.tile([P, NT], f32)
            nc.vector.tensor_tensor(out=jff[:], in0=jsel[:], in1=dfix[:], op=Alu.add)
            jfi = small.tile([P, NT], i32)
            nc.vector.tensor_copy(jfi[:], jff[:])

            gatf = small.tile([P, NT, D], f32)
            nc.gpsimd.indirect_dma_start(
                out=gatf[:], out_offset=None, in_=vrot[:],
                in_offset=bass.IndirectOffsetOnAxis(ap=jfi[:], axis=0),
            )

            # ---- exact f32 distance, koleo, clamp ----
            diff = small.tile([P, NT, D], f32)
            nc.vector.tensor_tensor(
                out=diff[:], in0=vr_sb[:], in1=gatf[:], op=Alu.subtract
            )
            diff2 = small.tile([P, NT, D], f32)
            nc.scalar.activation(diff2[:], diff[:], Act.Copy, bias=1.0e-6)
            dsq = small.tile([P, NT, D], f32)
            nc.vector.tensor_tensor(out=dsq[:], in0=diff2[:], in1=diff2[:], op=Alu.mult)
            s2 = small.tile([P, NT], f32)
            nc.vector.tensor_reduce(s2[:], dsq[:], axis=mybir.AxisListType.X, op=Alu.add)

            lns = small.tile([P, NT], f32)
            nc.scalar.activation(lns[:], s2[:], Act.Ln)
            kol = small.tile([P, NT], f32)
            nc.scalar.activation(
                kol[:], lns[:], Act.Copy, bias=-math.log(float(N)), scale=-0.5
            )
            if DEBUG_NO_CLAMP:
                nc.sync.dma_start(out_d[:], kol[:])
            else:
                kz = small.tile([P, NT], f32)
                nc.vector.tensor_scalar(
                    out=kz[:], in0=kol[:], scalar1=0.0, scalar2=None, op0=Alu.max
                )
                nc.sync.dma_start(out_d[:], kz[:])

    nc.compile()
    _built["nc"] = nc
    return nc


def _prep_in_maps(v: np.ndarray) -> list[dict]:
    bf = ml_dtypes.bfloat16
    in_maps = []
    for c in range(NCORES):
        vr = np.roll(v, -c * ROWS, axis=0)
        w = vr[0::2] + vr[1::2]  # [NPAIR, D] f32
        u = vr[0::2] - vr[1::2]
        rows = v[c * ROWS : (c + 1) * ROWS]
        wt = np.ascontiguousarray(w.T).astype(bf)
        ut = np.ascontiguousarray(u.T).astype(bf)
        rt = np.ascontiguousarray(rows.T).astype(bf)
        in_maps.append(
            {
                "wt": np.concatenate([wt, wt], axis=0),
                "ut": np.concatenate([ut, ut], axis=0),
                "vrows_t": np.concatenate([rt, rt], axis=0),
                "vrows_sb": np.ascontiguousarray(
                    rows.reshape(NT, P, D).transpose(1, 0, 2)
                ),
                "vrot": np.ascontiguousarray(vr),
            }
        )
    return in_maps


# test.py can flip these to profile the run
TRACE = False
DEBUG_NO_CLAMP = False
LAST_RESULT = {}


def kernel(latents: np.ndarray) -> np.ndarray:
    from concourse.bass_utils import run_bass_kernel_spmd

    v = np.asarray(latents, dtype=np.float32).reshape(N, D)
    nc = _build_nc()
    in_maps = _prep_in_maps(v)

    kwargs = {}
    if TRACE:
        kwargs = dict(trace=True, stitch_traces=False)
    res = run_bass_kernel_spmd(nc, in_maps, core_ids=list(range(NCORES)), **kwargs)
    LAST_RESULT["res"] = res

    vals = np.concatenate([r["out"].reshape(-1) for r in res.results])
    return np.array(np.mean(vals), dtype=np.float32)


# revision 13
# speedup vs baseline: 1.5533x; 1.1370x over previous
"""KoLeo loss kernel for 8 Trainium2 NeuronCores.

Reference semantics:
    v = latents.squeeze()            # [N, D] f32, N=16384, D=64
    dp = v @ v.T ; dp[i,i] = -1      # NxN scores, diagonal excluded
    idx = argmax(dp, axis=1)         # nearest neighbor by dot product
    dist = ||v - v[idx] + 1e-6||_2
    out = mean(relu(-log(dist * N)))

Sharding: rows are block-sharded 2048/core.  Each core gets a copy of v
whose rows are ROTATED by -core*2048, so the self-match diagonal of its
local [2048, 16384] score block always lands at pair-column (row//2) --
the SPMD program is identical on all cores.

Pairwise-max trick: max(a, b) = (a + b + |a-b|) / 2.  The host ships
paired sums w = v[0::2]+v[1::2] and diffs u = v[0::2]-v[1::2]; the PE
computes dp-sums s = rows @ w.T and dp-diffs d = rows @ u.T (same FLOPs
as plain dp).  ScalarE takes |d| out of PSUM (its only job), and a
fused custom VectorE op consumes (s from PSUM, |d| from SBUF) at one
output/cycle -- i.e. TWO dp elements per DVE cycle:

    pack = round_16384((s + |d|) * 8) + (pair_idx + 1);  accum = max

The fp32 magic-constant rounding makes the pack exact, so max over
packs == lexicographic (quantized pair-max, pair index) max.  The
diagonal is excluded exactly: accumulate -2^17 at the self position of
the SUM stream and -+2^17 (sign by parity) on the DIFF stream, which
turns the self-pair's max into its partner's value.

PE runs cold (1.2 GHz) on this part, so matmuls are row-packed with
tile_position: even row-tiles use array rows 0-63, odd tiles rows
64-127 (operands duplicated in SBUF partitions 64-127), two matmuls
concurrently in disjoint array quadrants.

Tail: recover the winning pair per row, gather both pair members with
batched indirect DMAs, pick the larger exact f32 dot (partner forced
when the winning pair is the self-pair), exact f32 distance, ln, clamp,
DMA out.  Host: mean of the 8x2048 per-row values.
"""

import math

import ml_dtypes
import numpy as np

N = 16384
D = 64
NCORES = 8
ROWS = N // NCORES  # 2048 rows per core
P = 128  # partitions
NT = ROWS // P  # 16 row-tiles per core
NPAIR = N // 2  # 8192 pair columns
CHUNK = 512  # pair columns per scan chunk (1 PSUM bank)
NCHUNK = NPAIR // CHUNK  # 16
BIG = 131072.0  # 2^17: diagonal suppression (exact in bf16, no f32 cancellation)

# fp32 pack constants: pack = round_16384((s+|d|)*8*16384) + pair_idx+1 (exact)
PACK_SCALE = 8.0 * 16384.0
PACK_MAGIC = 3.0 * 2.0**22 * 16384.0  # rounds to multiples of 16384
UNPACK_MAGIC = 12582912.0  # 3 * 2^22: rounds to integers

_OP_NAME = "KOLEO_PAIR_PACK_MAX"
_built = {}


def _register_pack_op():
    """Register the fused pair-max/argmax custom DVE op (idempotent)."""
    from concourse import dve_ops
    from concourse.dve_spec import (
        AluOp, C0, C1, One, Spec, Src0, Src1, Zero, lower, scan,
    )
    from concourse.dve_uop import DveOpSpec

    if _OP_NAME in dve_ops._SUB_OPCODE_FOR_NAME:
        return next(op for op in dve_ops.OPS if op.name == _OP_NAME)

    def _reference(in0, in1, s0, s1, imm2):
        p = in0.shape[0]
        s = in0.astype(np.float32).reshape(p, -1)
        a = in1.astype(np.float32).reshape(p, s.shape[1])
        z = ((s + a) * np.float32(s0) + np.float32(s1)) - np.float32(s1)
        col = (np.arange(s.shape[1], dtype=np.float32) + 1.0)[None, :]
        body = (z + col).astype(np.float32)
        acc = np.maximum(body.max(axis=-1, keepdims=True), 0.0)
        return body, acc

    body = ((Src0 + Src1) * C0 + C1 - C1) + scan(AluOp.ADD, One)
    spec = Spec(body=body, accum=AluOp.MAX, accum_init=Zero, reference=_reference)

    row = max(dve_ops._SUB_OPCODE_FOR_NAME.values()) + 1
    shas = {}
    for ver in ("v3", "v4"):
        uops = lower(spec, ver=ver)
        shas[ver] = DveOpSpec(
            name=_OP_NAME, opcode=row, uops=uops, rd1_en=True
        ).sha(ver)

    op = dve_ops.DveOp(_OP_NAME, spec, subdim=False, uops_sha=shas)
    dve_ops.OPS.append(op)
    dve_ops._SUB_OPCODE_FOR_NAME[_OP_NAME] = row
    dve_ops.CUSTOM_DVE_SPECS[_OP_NAME] = spec
    return op


def _build_nc():
    """Build + compile the per-core Bass program (same NEFF on all cores)."""
    if "nc" in _built:
        return _built["nc"]

    import concourse.bass as bass
    import concourse.mybir as mybir
    import concourse.tile as tile
    from concourse import bacc

    pack_op = _register_pack_op()

    f32 = mybir.dt.float32
    bf16 = mybir.dt.bfloat16
    i32 = mybir.dt.int32
    Alu = mybir.AluOpType
    Act = mybir.ActivationFunctionType

    nc = bacc.Bacc(None, target_bir_lowering=False)

    # w/u/vrows_t are duplicated into partitions 64-127 for row-packing
    wt_d = nc.declare_dram_parameter("wt", [P, NPAIR], bf16, isOutput=False)
    ut_d = nc.declare_dram_parameter("ut", [P, NPAIR], bf16, isOutput=False)
    vrows_t = nc.declare_dram_parameter("vrows_t", [P, ROWS], bf16, isOutput=False)
    # vrows duplicated along D so one op handles both gathered pair members
    vrows_sb = nc.declare_dram_parameter(
        "vrows_sb", [P, NT, 2 * D], f32, isOutput=False
    )
    # v viewed as pairs: row p holds v[2p] | v[2p+1] (one 512B gather per row)
    vpair = nc.declare_dram_parameter("vpair", [NPAIR, 2 * D], f32, isOutput=False)
    out_d = nc.declare_dram_parameter("out", [P, NT], f32, isOutput=True)

    neg_eye = nc.inline_tensor(
        (np.eye(P, dtype=np.float32) * -BIG).astype(ml_dtypes.bfloat16), "neg_eye"
    )
    sgn = np.where(np.arange(P) % 2 == 0, -BIG, BIG).astype(np.float32)
    alt_eye = nc.inline_tensor(
        (np.eye(P, dtype=np.float32) * sgn).astype(ml_dtypes.bfloat16), "alt_eye"
    )
    half_np = np.zeros((P, P // 2), dtype=np.float32)
    half_np[np.arange(P), np.arange(P) // 2] = 1.0
    half_eye = nc.inline_tensor(half_np.astype(ml_dtypes.bfloat16), "half_eye")
    iota_np = np.tile(np.arange(NCHUNK, dtype=np.float32), (P, NT))
    iota_c = nc.inline_tensor(iota_np, "iota_c")  # [P, NT*NCHUNK]
    rloc = (np.arange(NT)[None, :] * P + np.arange(P)[:, None]).astype(np.float32)
    selfpair_np = np.floor(rloc / 2.0)  # self pair index per row
    forcemem_np = 1.0 - (rloc % 2.0)  # partner member within the self pair
    selfpair_c = nc.inline_tensor(selfpair_np.astype(np.float32), "selfpair")
    forcemem_c = nc.inline_tensor(forcemem_np.astype(np.float32), "forcemem")

    with tile.TileContext(nc) as tc:
        with (
            tc.tile_pool(name="consts", bufs=1) as consts,
            tc.tile_pool(name="psum", bufs=2, space="PSUM") as psum_pool,
            tc.tile_pool(name="absp", bufs=4) as absp,
            tc.tile_pool(name="junk", bufs=2) as junk_pool,
            tc.tile_pool(name="small", bufs=1) as small,
        ):
            # ---- load inputs: small consts + stationaries first so chunk 0
            # (which carries the diagonal-mask matmuls) can start immediately
            negI_sb = consts.tile([P, P], bf16)
            nc.sync.dma_start(negI_sb[:], neg_eye[:])
            altI_sb = consts.tile([P, P], bf16)
            nc.sync.dma_start(altI_sb[:], alt_eye[:])
            halfI_sb = consts.tile([P, P // 2], bf16)
            nc.sync.dma_start(halfI_sb[:], half_eye[:])
            vrt_sb = consts.tile([P, ROWS], bf16)
            nc.sync.dma_start(vrt_sb[:], vrows_t[:])
            wt_sb = consts.tile([P, NPAIR], bf16)
            ut_sb = consts.tile([P, NPAIR], bf16)
            for c in range(0, NCHUNK, 4):
                sl = slice(c * CHUNK, (c + 4) * CHUNK)
                nc.sync.dma_start(wt_sb[:, sl], wt_d[:, sl])
                nc.sync.dma_start(ut_sb[:, sl], ut_d[:, sl])
            vr_sb = consts.tile([P, NT, 2 * D], f32)
            nc.sync.dma_start(vr_sb[:], vrows_sb[:])
            iota_sb = consts.tile([P, NT, NCHUNK], f32)
            nc.sync.dma_start(iota_sb[:], iota_c[:])
            selfpair_sb = consts.tile([P, NT], f32)
            nc.sync.dma_start(selfpair_sb[:], selfpair_c[:])
            forcemem_sb = consts.tile([P, NT], f32)
            nc.sync.dma_start(forcemem_sb[:], forcemem_c[:])

            bm = small.tile([P, NT, NCHUNK], f32)  # packed per-chunk maxima
            pff = small.tile([P, NT], f32)  # winning pair index (float)
            pfi = small.tile([P, NT], i32)  # winning pair index (int, for gather)
            gat01 = small.tile([P, NT, 2 * D], f32)  # gathered pair members
            g2 = small.tile([P, NT], f32)
            eqm = small.tile([P, NT, NCHUNK], f32)
            eqi = small.tile([P, NT, NCHUNK], f32)
            cstar = small.tile([P, NT], f32)
            u1 = small.tile([P, NT], f32)
            u2 = small.tile([P, NT], f32)
            idx1 = small.tile([P, NT], f32)
            ctl = small.tile([P, NT], f32)

            def winner_phase(s):
                """Recover pair s's winning pair index and issue its gather.

                Emitted right after pair s's scans: every input is already
                produced on the same engines, so nothing stalls; the gather
                descriptors + DMA land in the shadow of the next pair."""
                tA, tB = 2 * s, 2 * s + 1
                tsl = slice(tA, tB + 1)
                nc.vector.tensor_reduce(
                    g2[:, tsl], bm[:, tsl, :], axis=mybir.AxisListType.X, op=Alu.max
                )
                for k in (tA, tB):
                    nc.vector.tensor_scalar(
                        out=eqm[:, k, :], in0=bm[:, k, :],
                        scalar1=g2[:, k : k + 1], scalar2=None, op0=Alu.is_ge,
                    )
                nc.vector.tensor_tensor(
                    out=eqi[:, tsl, :], in0=eqm[:, tsl, :], in1=iota_sb[:, tsl, :],
                    op=Alu.mult,
                )
                nc.vector.tensor_reduce(
                    cstar[:, tsl], eqi[:, tsl, :], axis=mybir.AxisListType.X,
                    op=Alu.max,
                )
                # local pair index (+1): idx1 = g - 16384*round(g/16384)
                nc.scalar.activation(
                    u1[:, tsl], g2[:, tsl], Act.Copy,
                    bias=UNPACK_MAGIC, scale=1.0 / 16384.0,
                )
                nc.scalar.activation(
                    u2[:, tsl], u1[:, tsl], Act.Copy,
                    bias=16384.0 * UNPACK_MAGIC, scale=-16384.0,
                )
                nc.vector.tensor_tensor(
                    out=idx1[:, tsl], in0=g2[:, tsl], in1=u2[:, tsl], op=Alu.add
                )
                # pair index pf = cstar*CHUNK + (idx1 - 1)
                nc.scalar.activation(
                    ctl[:, tsl], cstar[:, tsl], Act.Copy, bias=-1.0,
                    scale=float(CHUNK),
                )
                nc.vector.tensor_tensor(
                    out=pff[:, tsl], in0=ctl[:, tsl], in1=idx1[:, tsl], op=Alu.add
                )
                nc.vector.tensor_copy(pfi[:, tsl], pff[:, tsl])
                nc.gpsimd.indirect_dma_start(
                    out=gat01[:, tsl, :], out_offset=None, in_=vpair[:],
                    in_offset=bass.IndirectOffsetOnAxis(ap=pfi[:, tsl], axis=0),
                )

            # ---- main loop: row-tile pairs (tA rows 0-63, tB rows 64-127) ----
            for s in range(NT // 2):
                tA, tB = 2 * s, 2 * s + 1
                lhsA = vrt_sb[0:64, tA * P : (tA + 1) * P]
                lhsB = vrt_sb[64:128, tB * P : (tB + 1) * P]
                for c in range(NCHUNK):
                    sl = slice(c * CHUNK, (c + 1) * CHUNK)
                    psA = psum_pool.tile([P, CHUNK], f32)
                    pdA = psum_pool.tile([P, CHUNK], f32)
                    psB = psum_pool.tile([P, CHUNK], f32)
                    pdB = psum_pool.tile([P, CHUNK], f32)
                    dA = c == tA // 8
                    dB = c == tB // 8
                    offA = (tA % 8) * 64
                    offB = (tB % 8) * 64
                    nc.tensor.matmul(
                        psA[:], lhsA, wt_sb[0:64, sl], start=True, stop=not dA
                    )
                    nc.tensor.matmul(
                        psB[:], lhsB, wt_sb[64:128, sl], start=True, stop=not dB
                    )
                    nc.tensor.matmul(
                        pdA[:], lhsA, ut_sb[0:64, sl], start=True, stop=not dA
                    )
                    nc.tensor.matmul(
                        pdB[:], lhsB, ut_sb[64:128, sl], start=True, stop=not dB
                    )
                    if dA:
                        nc.tensor.matmul(
                            psA[:, offA : offA + 64], negI_sb[:], halfI_sb[:],
                            start=False, stop=True,
                        )
                        nc.tensor.matmul(
                            pdA[:, offA : offA + 64], altI_sb[:], halfI_sb[:],
                            start=False, stop=True,
                        )
                    if dB:
                        nc.tensor.matmul(
                            psB[:, offB : offB + 64], negI_sb[:], halfI_sb[:],
                            start=False, stop=True,
                        )
                        nc.tensor.matmul(
                            pdB[:, offB : offB + 64], altI_sb[:], halfI_sb[:],
                            start=False, stop=True,
                        )
                    adA = absp.tile([P, CHUNK], f32)
                    nc.scalar.activation(adA[:], pdA[:], Act.Abs)
                    junkA = junk_pool.tile([P, CHUNK], f32)
                    nc.vector._custom_dve(
                        pack_op, out=junkA[:], in0=psA[:], in1=adA[:],
                        s0=PACK_SCALE, s1=PACK_MAGIC,
                        accum_out=bm[:, tA, c : c + 1],
                    )
                    adB = absp.tile([P, CHUNK], f32)
                    nc.scalar.activation(adB[:], pdB[:], Act.Abs)
                    junkB = junk_pool.tile([P, CHUNK], f32)
                    nc.vector._custom_dve(
                        pack_op, out=junkB[:], in0=psB[:], in1=adB[:],
                        s0=PACK_SCALE, s1=PACK_MAGIC,
                        accum_out=bm[:, tB, c : c + 1],
                    )
                winner_phase(s)

            # ---- both candidate distances + dots from the gathered pairs ----
            # gat01 rows: [v[2p] | v[2p+1]]; vr_sb rows: [v_r | v_r]
            d01 = small.tile([P, NT, 2, D], f32)
            nc.vector.tensor_tensor(
                out=d01[:], in0=vr_sb[:], in1=gat01[:], op=Alu.subtract
            )
            d01e = small.tile([P, NT, 2, D], f32)
            nc.scalar.activation(d01e[:], d01[:], Act.Copy, bias=1.0e-6)
            d01q = small.tile([P, NT, 2, D], f32)
            nc.vector.tensor_tensor(out=d01q[:], in0=d01e[:], in1=d01e[:], op=Alu.mult)
            s2q = small.tile([P, NT, 2], f32)
            nc.vector.tensor_reduce(
                s2q[:], d01q[:], axis=mybir.AxisListType.X, op=Alu.add
            )
            pr01 = small.tile([P, NT, 2, D], f32)
            nc.vector.tensor_tensor(out=pr01[:], in0=vr_sb[:], in1=gat01[:], op=Alu.mult)
            dotq = small.tile([P, NT, 2], f32)
            nc.vector.tensor_reduce(
                dotq[:], pr01[:], axis=mybir.AxisListType.X, op=Alu.add
            )

            # select member with larger dot; force partner on the self-pair
            sel = small.tile([P, NT], f32)  # 1.0 if member1 wins
            nc.vector.tensor_tensor(
                out=sel[:], in0=dotq[:, :, 1], in1=dotq[:, :, 0], op=Alu.is_gt
            )
            meq = small.tile([P, NT], f32)  # winning pair == self pair?
            nc.vector.tensor_tensor(
                out=meq[:], in0=pff[:], in1=selfpair_sb[:], op=Alu.is_equal
            )
            fdel = small.tile([P, NT], f32)
            nc.vector.tensor_tensor(
                out=fdel[:], in0=forcemem_sb[:], in1=sel[:], op=Alu.subtract
            )
            ffix = small.tile([P, NT], f32)
            nc.vector.tensor_tensor(out=ffix[:], in0=fdel[:], in1=meq[:], op=Alu.mult)
            self2 = small.tile([P, NT], f32)  # final member selector
            nc.vector.tensor_tensor(out=self2[:], in0=sel[:], in1=ffix[:], op=Alu.add)

            ds2 = small.tile([P, NT], f32)
            nc.vector.tensor_tensor(
                out=ds2[:], in0=s2q[:, :, 1], in1=s2q[:, :, 0], op=Alu.subtract
            )
            ds2s = small.tile([P, NT], f32)
            nc.vector.tensor_tensor(out=ds2s[:], in0=ds2[:], in1=self2[:], op=Alu.mult)
            s2 = small.tile([P, NT], f32)
            nc.vector.tensor_tensor(
                out=s2[:], in0=s2q[:, :, 0], in1=ds2s[:], op=Alu.add
            )

            lns = small.tile([P, NT], f32)
            nc.scalar.activation(lns[:], s2[:], Act.Ln)
            kol = small.tile([P, NT], f32)
            nc.scalar.activation(
                kol[:], lns[:], Act.Copy, bias=-math.log(float(N)), scale=-0.5
            )
            if DEBUG_NO_CLAMP:
                nc.sync.dma_start(out_d[:], kol[:])
            else:
                kz = small.tile([P, NT], f32)
                nc.vector.tensor_scalar(
                    out=kz[:], in0=kol[:], scalar1=0.0, scalar2=None, op0=Alu.max
                )
                nc.sync.dma_start(out_d[:], kz[:])

    nc.compile()
    _built["nc"] = nc
    return nc


def _prep_in_maps(v: np.ndarray) -> list[dict]:
    bf = ml_dtypes.bfloat16
    in_maps = []
    for c in range(NCORES):
        vr = np.roll(v, -c * ROWS, axis=0)
        w = vr[0::2] + vr[1::2]  # [NPAIR, D] f32
        u = vr[0::2] - vr[1::2]
        rows = v[c * ROWS : (c + 1) * ROWS]
        wt = np.ascontiguousarray(w.T).astype(bf)
        ut = np.ascontiguousarray(u.T).astype(bf)
        rt = np.ascontiguousarray(rows.T).astype(bf)
        rsb = rows.reshape(NT, P, D).transpose(1, 0, 2)
        in_maps.append(
            {
                "wt": np.concatenate([wt, wt], axis=0),
                "ut": np.concatenate([ut, ut], axis=0),
                "vrows_t": np.concatenate([rt, rt], axis=0),
                "vrows_sb": np.ascontiguousarray(
                    np.concatenate([rsb, rsb], axis=2)
                ),
                "vpair": np.ascontiguousarray(vr.reshape(NPAIR, 2 * D)),
            }
        )
    return in_maps


# test.py can flip these to profile the run
TRACE = False
DEBUG_NO_CLAMP = False
LAST_RESULT = {}


def kernel(latents: np.ndarray) -> np.ndarray:
    from concourse.bass_utils import run_bass_kernel_spmd

    v = np.asarray(latents, dtype=np.float32).reshape(N, D)
    nc = _build_nc()
    in_maps = _prep_in_maps(v)

    kwargs = {}
    if TRACE:
        kwargs = dict(trace=True, stitch_traces=False)
    res = run_bass_kernel_spmd(nc, in_maps, core_ids=list(range(NCORES)), **kwargs)
    LAST_RESULT["res"] = res

    vals = np.concatenate([r["out"].reshape(-1) for r in res.results])
    return np.array(np.mean(vals), dtype=np.float32)
